# revision 54
# baseline (speedup 1.0000x reference)
"""Trainium2 Bass kernel for nn_DecoderAttention (dual-key tree decoder attention).

Sharding: data-parallel over batch B=8, one batch element per NeuronCore.

Per-core computation (B-slice):
  q = target @ Wq + bq                     [T,F]   (kept transposed [F,T])
  k/v (node, leaf) = x @ {Wk,Wv}           (kept transposed, k and v stacked on
                                            the 128 partitions: rows 0:64 kT,
                                            rows 64:128 vT; bk cancels in the
                                            softmax so it is folded away, bv
                                            rides in the shared bias row)
  logits = leaf @ Wagg                     [L,1]   (PE 1-col matmuls off leafT;
                                            bagg cancels in the group softmax)
  Aqn/Aql softmaxes are computed unnormalized (exp, no max-subtraction: |scores/8| <~ 1.2)
  out_pre = (En^T @ [1|nh])/Z1 + (El^T @ [1|v])/Z2 + root/3
  out = softmax_F(out_pre)                 [T,F]
Both attentions accumulate t-major ([128 targets, 1|vals] PSUM regions, with the
exp tile as the stationary operand), so Z rides in column 0 and the final
softmax needs no transposes at all.
The tree interpolation's root term commutes through the suffix-mean and the
attention average (softmax weights sum to 1), so root/3 is added once at the end.
Suffix cumsum over L: per-128-chunk triangular matmuls (batched 4 chunks / matmul);
the cross-chunk carries are folded into the LAST ROW of each interp chunk before
the in-chunk suffix (row 127 participates in every suffix sum of its chunk).

All heavy matmuls run in bf16 (1 PE cycle/row vs 4 for fp32); accumulation stays
in fp32 PSUM. GPSIMD cannot touch PSUM, so PSUM drains go to ACT/DVE and gpsimd
keeps the SBUF-only elementwise work. The leaf loop is paced by the leaf DMA
stream and the attention phase by the ACT exp throughput.
"""

import os
import sys

import numpy as np

for _p in ("/opt/trn_rl_repo", "/root/.axon_site/_ro/trn_rl_repo"):
    if os.path.isdir(_p) and _p not in sys.path:
        sys.path.insert(0, _p)

import concourse.bass as bass
import concourse.tile as tile
from concourse import bacc
from concourse import mybir
from concourse.bass_utils import run_bass_kernel_spmd
from concourse.masks import make_identity, make_lower_triangular

FP = mybir.dt.float32
BF = mybir.dt.bfloat16
AF = mybir.ActivationFunctionType
OP = mybir.AluOpType
AX = mybir.AxisListType

B, T, N, L, D, F = 8, 1024, 512, 4096, 512, 64
BR = L // N          # 8 leaves per node
NC = L // 128        # 32 leaf chunks of 128
ND = D // 128        # 4 contraction chunks
SCALE = 1.0 / float(np.sqrt(F))


def _bcast_ap(ap, parts=128):
    """Partition-broadcast read AP (DRAM sources only)."""
    dims = list(ap.ap)
    if dims and dims[0][1] == 1:
        dims = dims[1:]
    return bass.AP(tensor=ap.tensor, offset=ap.offset, ap=[[0, parts]] + dims)


def _rep_ap(ap, rep):
    """Append a step-0 innermost free dim (read each element `rep` times)."""
    return bass.AP(tensor=ap.tensor, offset=ap.offset, ap=list(ap.ap) + [[0, rep]])


def build_nc():
    nc = bacc.Bacc("TRN2", target_bir_lowering=False, debug=False)

    d_root = nc.dram_tensor("root", [1, F], FP, kind="ExternalInput")
    d_node = nc.dram_tensor("node", [N, D], FP, kind="ExternalInput")
    d_leaf = nc.dram_tensor("leaf", [L, D], FP, kind="ExternalInput")
    d_target = nc.dram_tensor("target", [T, D], FP, kind="ExternalInput")
    d_wq = nc.dram_tensor("Wq", [D, F], FP, kind="ExternalInput")
    d_bq = nc.dram_tensor("bq", [F], FP, kind="ExternalInput")
    d_wk = nc.dram_tensor("Wk", [D, F], FP, kind="ExternalInput")
    d_bk = nc.dram_tensor("bk", [F], FP, kind="ExternalInput")
    d_wv = nc.dram_tensor("Wv", [D, F], FP, kind="ExternalInput")
    d_bv = nc.dram_tensor("bv", [F], FP, kind="ExternalInput")
    d_wagg = nc.dram_tensor("Wagg", [D, 1], FP, kind="ExternalInput")
    d_bagg = nc.dram_tensor("bagg", [1], FP, kind="ExternalInput")
    d_out = nc.dram_tensor("out", [T, F], FP, kind="ExternalOutput")

    with tile.TileContext(nc) as tc:
        _emit(nc, tc, d_root, d_node, d_leaf, d_target, d_wq, d_bq, d_wk, d_bk,
              d_wv, d_bv, d_wagg, d_bagg, d_out)
    nc.compile()
    return nc


def _emit(nc, tc, d_root, d_node, d_leaf, d_target, d_wq, d_bq, d_wk, d_bk,
          d_wv, d_bv, d_wagg, d_bagg, d_out):
    from contextlib import ExitStack

    with ExitStack() as ctx:
        consts = ctx.enter_context(tc.tile_pool(name="consts", bufs=1))
        big = ctx.enter_context(tc.tile_pool(name="big", bufs=1))
        lnat = ctx.enter_context(tc.tile_pool(name="lnat", bufs=4))
        lbf = ctx.enter_context(tc.tile_pool(name="lbf", bufs=2))
        ltp = ctx.enter_context(tc.tile_pool(name="ltp", bufs=2))
        work = ctx.enter_context(tc.tile_pool(name="work", bufs=2))
        epool = ctx.enter_context(tc.tile_pool(name="epool", bufs=32))
        enp = ctx.enter_context(tc.tile_pool(name="enp", bufs=4))
        ptr = ctx.enter_context(tc.tile_pool(name="ptr", bufs=2, space="PSUM"))
        pbig = ctx.enter_context(tc.tile_pool(name="pbig", bufs=2, space="PSUM"))
        pacc = ctx.enter_context(tc.tile_pool(name="pacc", bufs=1, space="PSUM"))

        # ---------------- constants ----------------
        ident = consts.tile([128, 128], FP)
        make_identity(nc, ident[:])
        ident_bf = consts.tile([128, 128], BF)
        make_identity(nc, ident_bf[:])
        tri128 = consts.tile([128, 128], BF)      # [m,l]=1 iff l<=m  (suffix-sum lhsT)
        make_lower_triangular(nc, tri128[:], val=1.0, diag=True)
        tri32s = consts.tile([32, 32], FP)        # [k,c]=1 iff k>c   (carry)
        make_lower_triangular(nc, tri32s[:], val=1.0, diag=False)

        # G[m,j] = 1 iff m//8 == j  (leaf->node group indicator), GT transposed
        G = consts.tile([128, 16], FP)
        nc.gpsimd.memset(G[:], 1.0)
        nc.gpsimd.affine_select(out=G[:], in_=G[:], compare_op=OP.is_ge, fill=0.0,
                                base=0, pattern=[[-BR, 16]], channel_multiplier=1)
        nc.gpsimd.affine_select(out=G[:], in_=G[:], compare_op=OP.is_ge, fill=0.0,
                                base=BR - 1, pattern=[[BR, 16]], channel_multiplier=-1)
        GT = consts.tile([16, 128], FP)
        nc.gpsimd.memset(GT[:], 1.0)
        nc.gpsimd.affine_select(out=GT[:], in_=GT[:], compare_op=OP.is_ge, fill=0.0,
                                base=0, pattern=[[1, 128]], channel_multiplier=-BR)
        nc.gpsimd.affine_select(out=GT[:], in_=GT[:], compare_op=OP.is_ge, fill=0.0,
                                base=BR - 1, pattern=[[-1, 128]], channel_multiplier=BR)

        # 1 / (3 * (L - l)) with l = 128*c + p   -> [128, 32]
        cnt3 = consts.tile([128, NC], FP)
        nc.gpsimd.iota(cnt3[:], pattern=[[-3 * 128, NC]], base=3 * L,
                       channel_multiplier=-3, allow_small_or_imprecise_dtypes=True)
        inv3 = consts.tile([128, NC], FP)
        nc.vector.reciprocal(inv3[:], cnt3[:])

        # ---------------- weights / biases ----------------
        # Every dma_start costs ~625ns of serialized HWDGE descriptor time, so
        # the loads the compute needs first go on the queue first: weights,
        # then node, then the leaf stream; target + the tiny bias/root loads
        # ride behind (their consumers run late).
        wagg_raw = consts.tile([128, ND], FP)
        bias_q = consts.tile([128, 1], FP)
        bias_kv = consts.tile([128, 1], FP)
        bq2 = d_bq[:].rearrange("(f o) -> f o", o=1)
        bk2 = d_bk[:].rearrange("(f o) -> f o", o=1)
        bv2 = d_bv[:].rearrange("(f o) -> f o", o=1)
        root_nat = consts.tile([128, F], FP)

        w_kv = consts.tile([128, ND, 128], BF)     # cols 0:64 Wk, 64:128 Wv per d-chunk
        w_q = consts.tile([128, ND, F], BF)
        wk_raw = consts.tile([128, ND, F], FP)
        wv_raw = consts.tile([128, ND, F], FP)
        wq_raw = consts.tile([128, ND, F], FP)
        nc.sync.dma_start(wk_raw[:], d_wk[:].rearrange("(j p) f -> p j f", p=128))
        nc.sync.dma_start(wv_raw[:], d_wv[:].rearrange("(j p) f -> p j f", p=128))
        nc.sync.dma_start(wq_raw[:], d_wq[:].rearrange("(j p) f -> p j f", p=128))
        for dc in range(ND):
            nc.vector.tensor_copy(w_kv[:, dc, 0:F], wk_raw[:, dc, :])
            nc.vector.tensor_copy(w_kv[:, dc, F:128], wv_raw[:, dc, :])
            nc.vector.tensor_copy(w_q[:, dc, :], wq_raw[:, dc, :])
        nc.sync.dma_start(wagg_raw[:], d_wagg[:].rearrange("(j p) o -> p (j o)", p=128))
        nc.sync.dma_start(bias_kv[0:F, :], bk2)
        nc.sync.dma_start(bias_kv[F:128, :], bv2)
        wagg16 = consts.tile([128, ND], BF)
        nc.vector.tensor_copy(wagg16[:], wagg_raw[:])
        root3_nat = consts.tile([128, F], FP)

        # ---------------- node -> kvtn [128, N] (kT rows 0:64, vT rows 64:128) ----
        nodeT = big.tile([128, ND, N], BF)
        nn = lnat.tile([128, 4, D], FP, tag="xnat")
        nc.sync.dma_start(nn[:], d_node[:].rearrange("(j p) d -> p j d", p=128))
        nb = lbf.tile([128, 4, D], BF, tag="xbf")
        nc.vector.tensor_copy(nb[:], nn[:])
        for jj in range(2):
            tp = ptr.tile([128, 1024], BF, tag="tp")
            for j2 in range(2):
                j = 2 * jj + j2
                for dc in range(ND):
                    nc.tensor.transpose(
                        tp[:, j2 * 512 + dc * 128:j2 * 512 + (dc + 1) * 128],
                        nb[:, j, dc * 128:(dc + 1) * 128], ident_bf[:])
            i0 = 2 * jj
            nc.vector.tensor_copy(
                nodeT[:, 0:ND, i0 * 128:(i0 + 2) * 128]
                .rearrange("p dc (j2 b) -> p j2 dc b", j2=2, b=128),
                tp[:].rearrange("p (j2 dc b) -> p j2 dc b", j2=2, dc=ND, b=128))
        kvtn = big.tile([128, N], BF)
        kvn_ps = pbig.tile([128, 1024], FP, tag="mm")
        for dc in range(ND):
            nc.tensor.matmul(kvn_ps[:, 0:512], w_kv[:, dc, :], nodeT[:, dc, :],
                             start=(dc == 0), stop=(dc == ND - 1))
        nc.scalar.activation(out=kvtn[:], in_=kvn_ps[:, 0:512], func=AF.Identity,
                             bias=bias_kv[:])

        # ---------------- target -> targT (bf16), qT [64, T] ----------------
        nc.sync.dma_start(bias_q[0:F, :], bq2)
        targT = big.tile([128, ND, T], BF)
        for ib in range(T // 512):
            tn = lnat.tile([128, 4, D], FP, tag="xnat")
            nc.sync.dma_start(tn[:], d_target[ib * 512:(ib + 1) * 512, :]
                              .rearrange("(j p) d -> p j d", p=128))
            tb = lbf.tile([128, 4, D], BF, tag="xbf")
            nc.vector.tensor_copy(tb[:], tn[:])
            for jj in range(2):
                tp = ptr.tile([128, 1024], BF, tag="tp")
                for j2 in range(2):
                    j = 2 * jj + j2
                    for dc in range(ND):
                        nc.tensor.transpose(
                            tp[:, j2 * 512 + dc * 128:j2 * 512 + (dc + 1) * 128],
                            tb[:, j, dc * 128:(dc + 1) * 128], ident_bf[:])
                i0 = 4 * ib + 2 * jj
                nc.vector.tensor_copy(
                    targT[:, 0:ND, i0 * 128:(i0 + 2) * 128]
                    .rearrange("p dc (j2 b) -> p j2 dc b", j2=2, b=128),
                    tp[:].rearrange("p (j2 dc b) -> p j2 dc b", j2=2, dc=ND, b=128))
        qT = big.tile([64, T], BF)
        for h in range(2):
            q_ps = pbig.tile([128, 1024], FP, tag="mm")
            for dc in range(ND):
                nc.tensor.matmul(q_ps[0:64, 0:512], w_q[:, dc, :],
                                 targT[:, dc, h * 512:(h + 1) * 512],
                                 start=(dc == 0), stop=(dc == ND - 1))
            nc.scalar.activation(out=qT[:, h * 512:(h + 1) * 512],
                                 in_=q_ps[0:64, 0:512], func=AF.Identity,
                                 bias=bias_q[0:F, :])

        # ---------------- leaf loop: leafT, kvt12, interpT, logits ----------------
        els = [None] * NC
        logits_nat = big.tile([128, NC], FP)   # raw leaf@Wagg logits (exp'd after loop)
        kvt12 = big.tile([128, L], BF)         # rows 0:64 leaf_kT, rows 64:128 leaf_vT
        interpT = big.tile([128, L], BF)       # rows 64:128: interp' = leaf_v + node_v
                                               # (kept on partitions 64:128 so the comb
                                               # transposes share one PE tile position)
        e_all = big.tile([128, NC], FP)        # exp(logits), natural chunk layout
        totT = big.tile([64, NC], FP)          # per-chunk interp totals (transposed)
        lns = []
        lbs = []

        def load_leaf(i):
            ln = lnat.tile([128, 4, D], FP, tag="xnat", name=f"ln{i}")
            nc.sync.dma_start(ln[:], d_leaf[i * 512:(i + 1) * 512, :]
                              .rearrange("(j p) d -> p j d", p=128))
            lb = lbf.tile([128, 4, D], BF, tag="xbf", name=f"lb{i}")
            nc.vector.tensor_copy(lb[:], ln[:])
            lns.append(ln)
            lbs.append(lb)

        load_leaf(0)
        for i in range(L // 512):
            # prefetch + convert the next block before this block's chain so the
            # ACT/DVE queues never head-block the convert behind iter-i work
            if i + 1 < L // 512:
                load_leaf(i + 1)
            lb = lbs[i]
            leafT = ltp.tile([128, ND, 512], BF)
            pl = pacc.tile([128, 4], FP, tag="acc", name=f"pl{i}")
            tps = []
            for jj in range(2):
                tp = ptr.tile([128, 1024], BF, tag="tp")
                for j2 in range(2):
                    j = 2 * jj + j2
                    for dc in range(ND):
                        nc.tensor.transpose(
                            tp[:, j2 * 512 + dc * 128:j2 * 512 + (dc + 1) * 128],
                            lb[:, j, dc * 128:(dc + 1) * 128], ident_bf[:])
                tps.append(tp)
            for jj in range(2):
                nc.vector.tensor_copy(
                    leafT[:, 0:ND, 2 * jj * 128:(2 * jj + 2) * 128]
                    .rearrange("p dc (j2 b) -> p j2 dc b", j2=2, b=128),
                    tps[jj][:].rearrange("p (j2 dc b) -> p j2 dc b", j2=2, dc=ND, b=128))
            kv_ps = pbig.tile([128, 1024], FP, tag="mm")
            for dc in range(ND):
                nc.tensor.matmul(kv_ps[:, 0:512], w_kv[:, dc, :], leafT[:, dc, :],
                                 start=(dc == 0), stop=(dc == ND - 1))
            for j in range(4):
                for dc in range(ND):
                    nc.tensor.matmul(pl[:, j:j + 1],
                                     leafT[:, dc, j * 128:(j + 1) * 128],
                                     wagg16[:, dc:dc + 1],
                                     start=(dc == 0), stop=(dc == ND - 1),
                                     skip_group_check=True)
            sl = slice(i * 512, (i + 1) * 512)
            nc.vector.tensor_copy(logits_nat[:, 4 * i:4 * i + 4], pl[:])
            nc.vector.tensor_scalar(out=kvt12[:, sl], in0=kv_ps[:, 0:512],
                                    scalar1=bias_kv[:], scalar2=None, op0=OP.add)
            # leaf attention scores + exp for this block's 4 chunks: ACT runs
            # only exps in the loop, so the exp stream paces it (~4.2us/iter)
            # and the 33us of exp work overlaps the leaf DMA stream. The last
            # two blocks' scores/exps are deferred into phase 2, whose crawl
            # otherwise leaves ACT idle.
            if i < 5:
                for j in range(4):
                    c = 4 * i + j
                    cs = slice(c * 128, (c + 1) * 128)
                    st2 = pbig.tile([128, 1024], FP, tag="mm", name=f"st{c}")
                    for h in range(2):
                        nc.tensor.matmul(st2[:, h * 512:(h + 1) * 512],
                                         kvt12[0:64, cs],
                                         qT[:, h * 512:(h + 1) * 512],
                                         start=True, stop=True)
                    el = epool.tile([128, 1024], BF, tag="el", name=f"el{c}")
                    nc.scalar.activation(out=el[:], in_=st2[:], func=AF.Exp,
                                         scale=SCALE)
                    els[c] = el
            # interp'T = leaf_vT + node_vT replicated 8x along l (no root, no /3)
            base = kvtn[64:128, 64 * i:64 * (i + 1)]
            nc.gpsimd.tensor_tensor(
                out=interpT[64:128, sl].rearrange("f (n c) -> f n c", c=BR),
                in0=kvt12[64:128, sl].rearrange("f (n c) -> f n c", c=BR),
                in1=_rep_ap(base, BR), op=OP.add)
            # per-chunk interp totals (for the carry) while the data is hot
            nc.vector.tensor_reduce(
                out=totT[:, 4 * i:4 * i + 4],
                in_=interpT[64:128, sl].rearrange("f (c m) -> f c m", m=128),
                axis=AX.X, op=OP.add)

        nc.scalar.activation(out=e_all[:], in_=logits_nat[:], func=AF.Exp)
        for c in range(20, NC):
            cs = slice(c * 128, (c + 1) * 128)
            st2 = pbig.tile([128, 1024], FP, tag="mm", name=f"st{c}")
            for h in range(2):
                nc.tensor.matmul(st2[:, h * 512:(h + 1) * 512],
                                 kvt12[0:64, cs],
                                 qT[:, h * 512:(h + 1) * 512],
                                 start=True, stop=True)
            el = epool.tile([128, 1024], BF, tag="el", name=f"el{c}")
            nc.scalar.activation(out=el[:], in_=st2[:], func=AF.Exp, scale=SCALE)
            els[c] = el
        # ---------------- node attention scores (exp kept, acc later) -------------
        en_t = []
        for b in range(4):
            st2 = pbig.tile([128, 1024], FP, tag="mm")
            for h in range(2):
                nc.tensor.matmul(st2[:, h * 512:(h + 1) * 512],
                                 kvtn[0:64, b * 128:(b + 1) * 128],
                                 qT[:, h * 512:(h + 1) * 512],
                                 start=True, stop=True)
            en = enp.tile([128, 1024], BF, tag="en", name=f"en{b}")
            nc.scalar.activation(out=en[:], in_=st2[:], func=AF.Exp, scale=SCALE)
            en_t.append(en)

        # ---------------- group-softmax weights over each node's leaf group -------
        s_ps = pbig.tile([16, NC], FP, tag="mm")
        nc.tensor.matmul(s_ps[:], G[:], e_all[:], start=True, stop=True)
        sinv = work.tile([16, NC], FP, tag="sinv")
        nc.vector.reciprocal(sinv[:], s_ps[:])
        r_ps = pbig.tile([128, NC], FP, tag="mm")
        nc.tensor.matmul(r_ps[:], GT[:], sinv[:], start=True, stop=True)
        w_all = work.tile([128, NC], FP, tag="w_all")
        nc.vector.tensor_tensor(out=w_all[:], in0=e_all[:], in1=r_ps[:], op=OP.mult)

        # ---------------- carry fold into last row of each interp chunk -----------
        tot_ps = ptr.tile([NC, 64], FP, tag="tp")
        nc.tensor.transpose(tot_ps[:], totT[:], ident[0:64, 0:64])
        totals = work.tile([NC, 64], FP, tag="tot")
        nc.vector.tensor_copy(totals[:], tot_ps[:])
        carrT_ps = ptr.tile([64, NC], FP, tag="tp")
        nc.tensor.matmul(carrT_ps[:], totals[:], tri32s[:], start=True, stop=True)
        # interpT[f, 128c+127] += carryT[f, c]  (row 127 is in every suffix sum)
        last_rows = interpT[64:128, 127::128]
        nc.vector.tensor_tensor(out=last_rows, in0=last_rows, in1=carrT_ps[:], op=OP.add)

        # One ACT-paced loop fuses: comb chunk build (PE transposes + DVE copy),
        # leaf attention scores+exp, the suffix-mean/node_hat machinery (rides in
        # the exp shadow), and the o2 accumulation (lags one chunk behind its exp).
        # wall32[:, c, :] holds w(l,c)*G placed at a 32-aligned half so chunk
        # pairs can accumulate node_hat at legal PE tile positions with no
        # in-loop pool work.
        comb = big.tile([128, NC, 129], BF)
        nc.gpsimd.memset(comb[:, :, 0:1], 1.0)
        nh_nat = big.tile([128, 4, 65], BF)    # [1 | nh] per node-chunk
        nc.gpsimd.memset(nh_nat[:, :, 0:1], 1.0)
        wall32 = big.tile([128, NC, 32], BF)
        nc.gpsimd.memset(wall32[:], 0.0)
        for c in range(NC):
            o16 = 16 * (c % 2)
            nc.gpsimd.tensor_scalar(out=wall32[:, c, o16:o16 + 16],
                                    in0=G[:], scalar1=w_all[:, c:c + 1],
                                    scalar2=None, op0=OP.mult)
        # o2T accumulates t-major: for each 128-target block k, region
        # [:, k//4, (k%4)*128 : +65] holds [Z2 | o2 vals] with t on partitions.
        o2t_ps = pacc.tile([128, 2, 512], FP, tag="acc", name="o2t_ps")
        for c in range(NC):
            cs = slice(c * 128, (c + 1) * 128)
            tpc = ptr.tile([128, 1024], BF, tag="tp")
            nc.tensor.transpose(tpc[:, 0:64], kvt12[64:128, cs],
                                ident_bf[64:128, 64:128])
            nc.tensor.transpose(tpc[:, 64:128], interpT[64:128, cs],
                                ident_bf[64:128, 64:128])
            nc.vector.tensor_copy(comb[:, c, 1:129], tpc[:, 0:128])
            if c % 4 == 3:
                # suffix-mean for chunks 4c4..4c4+3, then their node_hat partials
                # accumulated in the same PSUM tile (cols 256:320)
                c4 = c // 4
                sfx = pbig.tile([128, 1024], FP, tag="mm", name=f"sfx{c4}")
                nc.tensor.matmul(sfx[:, 0:256].rearrange("p (cc f) -> p cc f", f=64),
                                 tri128[:], comb[:, 4 * c4:4 * c4 + 4, 65:129],
                                 start=True, stop=True)
                upw4 = work.tile([128, 4, 64], BF, tag="upw")
                nc.vector.tensor_tensor(
                    out=upw4[:],
                    in0=sfx[:, 0:256].rearrange("p (cc f) -> p cc f", f=64),
                    in1=_rep_ap(inv3[:, 4 * c4:4 * c4 + 4], 64), op=OP.mult)
                for jc in range(4):
                    cc = 4 * c4 + jc
                    po = 32 * (jc // 2)
                    nc.tensor.matmul(sfx[po:po + 32, 256:320], wall32[:, cc, :],
                                     upw4[:, jc, :],
                                     start=(jc % 2 == 0), stop=(jc % 2 == 1),
                                     skip_group_check=True)
                g, ghalf = c4 // 2, c4 % 2
                nc.vector.tensor_copy(nh_nat[64 * ghalf:64 * ghalf + 64, g, 1:65],
                                      sfx[0:64, 256:320])
            for k in range(T // 128):
                nc.tensor.matmul(
                    o2t_ps[:, k // 4, (k % 4) * 128:(k % 4) * 128 + 65],
                    els[c][:, k * 128:(k + 1) * 128],
                    comb[:, c, 0:65],
                    start=(c == 0), stop=(c == NC - 1), skip_group_check=True)

        # ---------------- node attention accumulate, t-major ----------------------
        # ptr's two transpose slots are free after the merged loop; they hold
        # the two 4-block halves of o1T.
        o1t = [ptr.tile([128, 512], FP, tag="tp", name=f"o1t{a}") for a in range(2)]
        for b in range(4):
            for k in range(T // 128):
                nc.tensor.matmul(o1t[k // 4][:, (k % 4) * 128:(k % 4) * 128 + 65],
                                 en_t[b][:, k * 128:(k + 1) * 128],
                                 nh_nat[:, b, 0:65],
                                 start=(b == 0), stop=(b == 3),
                                 skip_group_check=True)

        # ---------------- combine + final softmax over F, t-major -----------------
        nc.sync.dma_start(root_nat[:], _bcast_ap(d_root[:].rearrange("o f -> (o f)")))
        nc.vector.tensor_scalar(out=root3_nat[:], in0=root_nat[:],
                                scalar1=1.0 / 3.0, scalar2=None, op0=OP.mult)

        def _oview(t, off, n):
            # [128, 8 blocks, n] strided view of the per-t-block regions
            return bass.AP(tensor=t.tensor, offset=t.offset + off,
                           ap=[list(t.ap[0])] + [[512, 2], [128, 4], [1, n]])

        def _rep_mid(ap, rep):
            # [128, rep, ...] view with a step-0 block dim after the partition
            return bass.AP(tensor=ap.tensor, offset=ap.offset,
                           ap=[list(ap.ap[0])] + [[0, rep]] + list(ap.ap)[1:])

        rz1 = work.tile([128, 8], FP, tag="rz1")
        nc.vector.reciprocal(rz1[:].rearrange("p (a b o) -> p a b o", a=2, o=1),
                             _oview(o2t_ps[:], 0, 1))
        def _hview(t, off, n):
            # [128, 4 blocks, n] strided view within one o1t half
            return bass.AP(tensor=t.tensor, offset=t.offset + off,
                           ap=[list(t.ap[0])] + [[128, 4], [1, n]])

        rz2 = work.tile([128, 8], FP, tag="rz2")
        for a in range(2):
            nc.vector.reciprocal(rz2[:, 4 * a:4 * a + 4]
                                 .rearrange("p (b o) -> p b o", o=1),
                                 _hview(o1t[a][:], 0, 1))
        s12all = big.tile([128, 8, F], FP)
        x2all = big.tile([128, 8, F], FP)
        nc.vector.tensor_tensor(out=s12all[:], in0=_oview(o2t_ps[:], 1, 64),
                                in1=_rep_ap(rz1[:], F), op=OP.mult)
        for a in range(2):
            nc.vector.tensor_tensor(out=x2all[:, 4 * a:4 * a + 4, :],
                                    in0=_hview(o1t[a][:], 1, 64),
                                    in1=_rep_ap(rz2[:, 4 * a:4 * a + 4], F),
                                    op=OP.mult)
        nc.gpsimd.tensor_tensor(out=s12all[:], in0=s12all[:], in1=x2all[:], op=OP.add)
        nc.gpsimd.tensor_tensor(out=s12all[:], in0=s12all[:],
                                in1=_rep_mid(root3_nat[:], 8), op=OP.add)
        e3a = big.tile([128, 8, F], FP)
        nc.scalar.activation(out=e3a[:], in_=s12all[:], func=AF.Exp)
        z8 = work.tile([128, 8], FP, tag="z8")
        nc.vector.tensor_reduce(out=z8[:], in_=e3a[:], axis=AX.X, op=OP.add)
        rz = work.tile([128, 8], FP, tag="rz")
        nc.vector.reciprocal(rz[:], z8[:])
        onat = big.tile([128, 8, F], FP)
        nc.vector.tensor_tensor(out=onat[:], in0=e3a[:], in1=_rep_ap(rz[:], F),
                                op=OP.mult)
        nc.sync.dma_start(d_out[:].rearrange("(k p) f -> p k f", p=128), onat[:])


_NC_CACHE = None


def kernel(**inputs):
    global _NC_CACHE
    if _NC_CACHE is None:
        _NC_CACHE = build_nc()
    nc = _NC_CACHE
    shared = {k: np.ascontiguousarray(np.asarray(inputs[k], dtype=np.float32))
              for k in ("Wq", "bq", "Wk", "bk", "Wv", "bv", "Wagg", "bagg")}
    in_maps = []
    for b in range(B):
        m = dict(shared)
        m["root"] = np.ascontiguousarray(np.asarray(inputs["root"][b], dtype=np.float32))
        m["node"] = np.ascontiguousarray(np.asarray(inputs["node"][b], dtype=np.float32))
        m["leaf"] = np.ascontiguousarray(np.asarray(inputs["leaf"][b], dtype=np.float32))
        m["target"] = np.ascontiguousarray(np.asarray(inputs["target"][b], dtype=np.float32))
        in_maps.append(m)
    res = run_bass_kernel_spmd(nc, in_maps, core_ids=list(range(B)))
    return np.stack([r["out"] for r in res.results], axis=0)


# revision 57
# speedup vs baseline: 1.0096x; 1.0096x over previous
"""Trainium2 Bass kernel for nn_DecoderAttention (dual-key tree decoder attention).

Sharding: data-parallel over batch B=8, one batch element per NeuronCore.

Per-core computation (B-slice):
  q = target @ Wq + bq                     [T,F]   (kept transposed [F,T])
  k/v (node, leaf) = x @ {Wk,Wv}           (kept transposed, k and v stacked on
                                            the 128 partitions: rows 0:64 kT,
                                            rows 64:128 vT; bk cancels in the
                                            softmax so it is folded away, bv
                                            rides in the shared bias row)
  logits = leaf @ Wagg                     [L,1]   (PE 1-col matmuls off leafT;
                                            bagg cancels in the group softmax)
  Aqn/Aql softmaxes are computed unnormalized (exp, no max-subtraction: |scores/8| <~ 1.2)
  out_pre = (En^T @ [1|nh])/Z1 + (El^T @ [1|v])/Z2 + root/3
  out = softmax_F(out_pre)                 [T,F]
Both attentions accumulate t-major ([128 targets, 1|vals] PSUM regions, with the
exp tile as the stationary operand), so Z rides in column 0 and the final
softmax needs no transposes at all.
The tree interpolation's root term commutes through the suffix-mean and the
attention average (softmax weights sum to 1), so root/3 is added once at the end.
Suffix cumsum over L: per-128-chunk triangular matmuls (batched 4 chunks / matmul);
the cross-chunk carries are folded into the LAST ROW of each interp chunk before
the in-chunk suffix (row 127 participates in every suffix sum of its chunk).

All heavy matmuls run in bf16 (1 PE cycle/row vs 4 for fp32); accumulation stays
in fp32 PSUM. GPSIMD cannot touch PSUM, so PSUM drains go to ACT/DVE and gpsimd
keeps the SBUF-only elementwise work. The leaf loop is paced by the leaf DMA
stream and the attention phase by the ACT exp throughput.
"""

import os
import sys

import numpy as np

for _p in ("/opt/trn_rl_repo", "/root/.axon_site/_ro/trn_rl_repo"):
    if os.path.isdir(_p) and _p not in sys.path:
        sys.path.insert(0, _p)

import concourse.bass as bass
import concourse.tile as tile
from concourse import bacc
from concourse import mybir
from concourse.bass_utils import run_bass_kernel_spmd
from concourse.masks import make_identity, make_lower_triangular

FP = mybir.dt.float32
BF = mybir.dt.bfloat16
AF = mybir.ActivationFunctionType
OP = mybir.AluOpType
AX = mybir.AxisListType

B, T, N, L, D, F = 8, 1024, 512, 4096, 512, 64
BR = L // N          # 8 leaves per node
NC = L // 128        # 32 leaf chunks of 128
ND = D // 128        # 4 contraction chunks
SCALE = 1.0 / float(np.sqrt(F))


def _bcast_ap(ap, parts=128):
    """Partition-broadcast read AP (DRAM sources only)."""
    dims = list(ap.ap)
    if dims and dims[0][1] == 1:
        dims = dims[1:]
    return bass.AP(tensor=ap.tensor, offset=ap.offset, ap=[[0, parts]] + dims)


def _rep_ap(ap, rep):
    """Append a step-0 innermost free dim (read each element `rep` times)."""
    return bass.AP(tensor=ap.tensor, offset=ap.offset, ap=list(ap.ap) + [[0, rep]])


def build_nc():
    nc = bacc.Bacc("TRN2", target_bir_lowering=False, debug=False)

    d_root = nc.dram_tensor("root", [1, F], FP, kind="ExternalInput")
    d_node = nc.dram_tensor("node", [N, D], FP, kind="ExternalInput")
    d_leaf = nc.dram_tensor("leaf", [L, D], FP, kind="ExternalInput")
    d_target = nc.dram_tensor("target", [T, D], FP, kind="ExternalInput")
    d_wq = nc.dram_tensor("Wq", [D, F], FP, kind="ExternalInput")
    d_bq = nc.dram_tensor("bq", [F], FP, kind="ExternalInput")
    d_wk = nc.dram_tensor("Wk", [D, F], FP, kind="ExternalInput")
    d_bk = nc.dram_tensor("bk", [F], FP, kind="ExternalInput")
    d_wv = nc.dram_tensor("Wv", [D, F], FP, kind="ExternalInput")
    d_bv = nc.dram_tensor("bv", [F], FP, kind="ExternalInput")
    d_wagg = nc.dram_tensor("Wagg", [D, 1], FP, kind="ExternalInput")
    d_bagg = nc.dram_tensor("bagg", [1], FP, kind="ExternalInput")
    d_out = nc.dram_tensor("out", [T, F], FP, kind="ExternalOutput")

    with tile.TileContext(nc) as tc:
        _emit(nc, tc, d_root, d_node, d_leaf, d_target, d_wq, d_bq, d_wk, d_bk,
              d_wv, d_bv, d_wagg, d_bagg, d_out)
    nc.compile()
    return nc


def _emit(nc, tc, d_root, d_node, d_leaf, d_target, d_wq, d_bq, d_wk, d_bk,
          d_wv, d_bv, d_wagg, d_bagg, d_out):
    from contextlib import ExitStack

    with ExitStack() as ctx:
        consts = ctx.enter_context(tc.tile_pool(name="consts", bufs=1))
        big = ctx.enter_context(tc.tile_pool(name="big", bufs=1))
        lnat = ctx.enter_context(tc.tile_pool(name="lnat", bufs=4))
        lbf = ctx.enter_context(tc.tile_pool(name="lbf", bufs=2))
        ltp = ctx.enter_context(tc.tile_pool(name="ltp", bufs=2))
        work = ctx.enter_context(tc.tile_pool(name="work", bufs=2))
        epool = ctx.enter_context(tc.tile_pool(name="epool", bufs=32))
        enp = ctx.enter_context(tc.tile_pool(name="enp", bufs=4))
        ptr = ctx.enter_context(tc.tile_pool(name="ptr", bufs=2, space="PSUM"))
        pbig = ctx.enter_context(tc.tile_pool(name="pbig", bufs=2, space="PSUM"))
        pacc = ctx.enter_context(tc.tile_pool(name="pacc", bufs=1, space="PSUM"))

        # ---------------- constants ----------------
        ident = consts.tile([128, 128], FP)
        make_identity(nc, ident[:])
        ident_bf = consts.tile([128, 128], BF)
        make_identity(nc, ident_bf[:])
        tri128 = consts.tile([128, 128], BF)      # [m,l]=1 iff l<=m  (suffix-sum lhsT)
        make_lower_triangular(nc, tri128[:], val=1.0, diag=True)
        tri32s = consts.tile([32, 32], FP)        # [k,c]=1 iff k>c   (carry)
        make_lower_triangular(nc, tri32s[:], val=1.0, diag=False)

        # G[m,j] = 1 iff m//8 == j  (leaf->node group indicator), GT transposed
        G = consts.tile([128, 16], FP)
        nc.gpsimd.memset(G[:], 1.0)
        nc.gpsimd.affine_select(out=G[:], in_=G[:], compare_op=OP.is_ge, fill=0.0,
                                base=0, pattern=[[-BR, 16]], channel_multiplier=1)
        nc.gpsimd.affine_select(out=G[:], in_=G[:], compare_op=OP.is_ge, fill=0.0,
                                base=BR - 1, pattern=[[BR, 16]], channel_multiplier=-1)
        GT = consts.tile([16, 128], FP)
        nc.gpsimd.memset(GT[:], 1.0)
        nc.gpsimd.affine_select(out=GT[:], in_=GT[:], compare_op=OP.is_ge, fill=0.0,
                                base=0, pattern=[[1, 128]], channel_multiplier=-BR)
        nc.gpsimd.affine_select(out=GT[:], in_=GT[:], compare_op=OP.is_ge, fill=0.0,
                                base=BR - 1, pattern=[[-1, 128]], channel_multiplier=BR)

        # 1 / (3 * (L - l)) with l = 128*c + p   -> [128, 32]
        cnt3 = consts.tile([128, NC], FP)
        nc.gpsimd.iota(cnt3[:], pattern=[[-3 * 128, NC]], base=3 * L,
                       channel_multiplier=-3, allow_small_or_imprecise_dtypes=True)
        inv3 = consts.tile([128, NC], FP)
        nc.vector.reciprocal(inv3[:], cnt3[:])

        # ---------------- weights / biases ----------------
        # Every dma_start costs ~625ns of serialized HWDGE descriptor time, so
        # the loads the compute needs first go on the queue first: weights,
        # then node, then the leaf stream; target + the tiny bias/root loads
        # ride behind (their consumers run late).
        wagg_raw = consts.tile([128, ND], FP)
        bias_q = consts.tile([128, 1], FP)
        bias_kv = consts.tile([128, 1], FP)
        bq2 = d_bq[:].rearrange("(f o) -> f o", o=1)
        bk2 = d_bk[:].rearrange("(f o) -> f o", o=1)
        bv2 = d_bv[:].rearrange("(f o) -> f o", o=1)
        root_nat = consts.tile([128, F], FP)

        w_kv = consts.tile([128, ND, 128], BF)     # cols 0:64 Wk, 64:128 Wv per d-chunk
        w_q = consts.tile([128, ND, F], BF)
        wk_raw = consts.tile([128, ND, F], FP)
        wv_raw = consts.tile([128, ND, F], FP)
        wq_raw = consts.tile([128, ND, F], FP)
        nc.sync.dma_start(wk_raw[:], d_wk[:].rearrange("(j p) f -> p j f", p=128))
        nc.sync.dma_start(wv_raw[:], d_wv[:].rearrange("(j p) f -> p j f", p=128))
        nc.sync.dma_start(wq_raw[:], d_wq[:].rearrange("(j p) f -> p j f", p=128))
        for dc in range(ND):
            nc.vector.tensor_copy(w_kv[:, dc, 0:F], wk_raw[:, dc, :])
            nc.vector.tensor_copy(w_kv[:, dc, F:128], wv_raw[:, dc, :])
            nc.vector.tensor_copy(w_q[:, dc, :], wq_raw[:, dc, :])
        nc.sync.dma_start(wagg_raw[:], d_wagg[:].rearrange("(j p) o -> p (j o)", p=128))
        nc.sync.dma_start(bias_kv[0:F, :], bk2)
        nc.sync.dma_start(bias_kv[F:128, :], bv2)
        wagg16 = consts.tile([128, ND], BF)
        nc.vector.tensor_copy(wagg16[:], wagg_raw[:])
        root3_nat = consts.tile([128, F], FP)

        # ---------------- node -> kvtn [128, N] (kT rows 0:64, vT rows 64:128) ----
        nodeT = big.tile([128, ND, N], BF)
        nn = lnat.tile([128, 4, D], FP, tag="xnat")
        nc.sync.dma_start(nn[:], d_node[:].rearrange("(j p) d -> p j d", p=128))
        nb = lbf.tile([128, 4, D], BF, tag="xbf")
        nc.vector.tensor_copy(nb[:], nn[:])
        for jj in range(2):
            tp = ptr.tile([128, 1024], BF, tag="tp")
            for j2 in range(2):
                j = 2 * jj + j2
                for dc in range(ND):
                    nc.tensor.transpose(
                        tp[:, j2 * 512 + dc * 128:j2 * 512 + (dc + 1) * 128],
                        nb[:, j, dc * 128:(dc + 1) * 128], ident_bf[:])
            i0 = 2 * jj
            nc.vector.tensor_copy(
                nodeT[:, 0:ND, i0 * 128:(i0 + 2) * 128]
                .rearrange("p dc (j2 b) -> p j2 dc b", j2=2, b=128),
                tp[:].rearrange("p (j2 dc b) -> p j2 dc b", j2=2, dc=ND, b=128))
        kvtn = big.tile([128, N], BF)
        kvn_ps = pbig.tile([128, 1024], FP, tag="mm")
        for dc in range(ND):
            nc.tensor.matmul(kvn_ps[:, 0:512], w_kv[:, dc, :], nodeT[:, dc, :],
                             start=(dc == 0), stop=(dc == ND - 1))
        nc.scalar.activation(out=kvtn[:], in_=kvn_ps[:, 0:512], func=AF.Identity,
                             bias=bias_kv[:])

        # ---------------- target -> targT (bf16), qT [64, T] ----------------
        nc.sync.dma_start(bias_q[0:F, :], bq2)
        targT = big.tile([128, ND, T], BF)
        for ib in range(T // 512):
            tn = lnat.tile([128, 4, D], FP, tag="xnat")
            nc.sync.dma_start(tn[:], d_target[ib * 512:(ib + 1) * 512, :]
                              .rearrange("(j p) d -> p j d", p=128))
            tb = lbf.tile([128, 4, D], BF, tag="xbf")
            nc.vector.tensor_copy(tb[:], tn[:])
            for jj in range(2):
                tp = ptr.tile([128, 1024], BF, tag="tp")
                for j2 in range(2):
                    j = 2 * jj + j2
                    for dc in range(ND):
                        nc.tensor.transpose(
                            tp[:, j2 * 512 + dc * 128:j2 * 512 + (dc + 1) * 128],
                            tb[:, j, dc * 128:(dc + 1) * 128], ident_bf[:])
                i0 = 4 * ib + 2 * jj
                nc.vector.tensor_copy(
                    targT[:, 0:ND, i0 * 128:(i0 + 2) * 128]
                    .rearrange("p dc (j2 b) -> p j2 dc b", j2=2, b=128),
                    tp[:].rearrange("p (j2 dc b) -> p j2 dc b", j2=2, dc=ND, b=128))
        qT = big.tile([64, T], BF)
        for h in range(2):
            q_ps = pbig.tile([128, 1024], FP, tag="mm")
            for dc in range(ND):
                nc.tensor.matmul(q_ps[0:64, 0:512], w_q[:, dc, :],
                                 targT[:, dc, h * 512:(h + 1) * 512],
                                 start=(dc == 0), stop=(dc == ND - 1))
            nc.scalar.activation(out=qT[:, h * 512:(h + 1) * 512],
                                 in_=q_ps[0:64, 0:512], func=AF.Identity,
                                 bias=bias_q[0:F, :])

        # ---------------- leaf loop: leafT, kvt12, interpT, logits ----------------
        els = [None] * NC
        logits_nat = big.tile([128, NC], FP)   # raw leaf@Wagg logits (exp'd after loop)
        kvt12 = big.tile([128, L], BF)         # rows 0:64 leaf_kT, rows 64:128 leaf_vT
        interpT = big.tile([128, L], BF)       # rows 64:128: interp' = leaf_v + node_v
                                               # (kept on partitions 64:128 so the comb
                                               # transposes share one PE tile position)
        e_all = big.tile([128, NC], FP)        # exp(logits), natural chunk layout
        totT = big.tile([64, NC], FP)          # per-chunk interp totals (transposed)
        lns = []
        lbs = []

        def load_leaf(i):
            ln = lnat.tile([128, 4, D], FP, tag="xnat", name=f"ln{i}")
            nc.sync.dma_start(ln[:], d_leaf[i * 512:(i + 1) * 512, :]
                              .rearrange("(j p) d -> p j d", p=128))
            lb = lbf.tile([128, 4, D], BF, tag="xbf", name=f"lb{i}")
            nc.vector.tensor_copy(lb[:], ln[:])
            lns.append(ln)
            lbs.append(lb)

        load_leaf(0)
        for i in range(L // 512):
            # prefetch + convert the next block before this block's chain so the
            # ACT/DVE queues never head-block the convert behind iter-i work
            if i + 1 < L // 512:
                load_leaf(i + 1)
            lb = lbs[i]
            leafT = ltp.tile([128, ND, 512], BF)
            pl = pacc.tile([128, 4], FP, tag="acc", name=f"pl{i}")
            tps = []
            for jj in range(2):
                tp = ptr.tile([128, 1024], BF, tag="tp")
                for j2 in range(2):
                    j = 2 * jj + j2
                    for dc in range(ND):
                        nc.tensor.transpose(
                            tp[:, j2 * 512 + dc * 128:j2 * 512 + (dc + 1) * 128],
                            lb[:, j, dc * 128:(dc + 1) * 128], ident_bf[:])
                tps.append(tp)
            for jj in range(2):
                nc.vector.tensor_copy(
                    leafT[:, 0:ND, 2 * jj * 128:(2 * jj + 2) * 128]
                    .rearrange("p dc (j2 b) -> p j2 dc b", j2=2, b=128),
                    tps[jj][:].rearrange("p (j2 dc b) -> p j2 dc b", j2=2, dc=ND, b=128))
            kv_ps = pbig.tile([128, 1024], FP, tag="mm")
            for dc in range(ND):
                nc.tensor.matmul(kv_ps[:, 0:512], w_kv[:, dc, :], leafT[:, dc, :],
                                 start=(dc == 0), stop=(dc == ND - 1))
            for j in range(4):
                for dc in range(ND):
                    nc.tensor.matmul(pl[:, j:j + 1],
                                     leafT[:, dc, j * 128:(j + 1) * 128],
                                     wagg16[:, dc:dc + 1],
                                     start=(dc == 0), stop=(dc == ND - 1),
                                     skip_group_check=True)
            sl = slice(i * 512, (i + 1) * 512)
            nc.vector.tensor_copy(logits_nat[:, 4 * i:4 * i + 4], pl[:])
            nc.vector.tensor_scalar(out=kvt12[:, sl], in0=kv_ps[:, 0:512],
                                    scalar1=bias_kv[:], scalar2=None, op0=OP.add)
            # leaf attention scores + exp for this block's 4 chunks: ACT runs
            # only exps in the loop, so the exp stream paces it (~4.2us/iter)
            # and the 33us of exp work overlaps the leaf DMA stream. The last
            # two blocks' scores/exps are deferred into phase 2, whose crawl
            # otherwise leaves ACT idle.
            for j in range(4):
                c = 4 * i + j
                if c < 22:
                    cs = slice(c * 128, (c + 1) * 128)
                    st2 = pbig.tile([128, 1024], FP, tag="mm", name=f"st{c}")
                    for h in range(2):
                        nc.tensor.matmul(st2[:, h * 512:(h + 1) * 512],
                                         kvt12[0:64, cs],
                                         qT[:, h * 512:(h + 1) * 512],
                                         start=True, stop=True)
                    el = epool.tile([128, 1024], BF, tag="el", name=f"el{c}")
                    nc.scalar.activation(out=el[:], in_=st2[:], func=AF.Exp,
                                         scale=SCALE)
                    els[c] = el
            # interp'T = leaf_vT + node_vT replicated 8x along l (no root, no /3)
            base = kvtn[64:128, 64 * i:64 * (i + 1)]
            nc.gpsimd.tensor_tensor(
                out=interpT[64:128, sl].rearrange("f (n c) -> f n c", c=BR),
                in0=kvt12[64:128, sl].rearrange("f (n c) -> f n c", c=BR),
                in1=_rep_ap(base, BR), op=OP.add)
            # per-chunk interp totals (for the carry) while the data is hot
            nc.vector.tensor_reduce(
                out=totT[:, 4 * i:4 * i + 4],
                in_=interpT[64:128, sl].rearrange("f (c m) -> f c m", m=128),
                axis=AX.X, op=OP.add)

        nc.scalar.activation(out=e_all[:], in_=logits_nat[:], func=AF.Exp)
        for c in range(22, NC):
            cs = slice(c * 128, (c + 1) * 128)
            st2 = pbig.tile([128, 1024], FP, tag="mm", name=f"st{c}")
            for h in range(2):
                nc.tensor.matmul(st2[:, h * 512:(h + 1) * 512],
                                 kvt12[0:64, cs],
                                 qT[:, h * 512:(h + 1) * 512],
                                 start=True, stop=True)
            el = epool.tile([128, 1024], BF, tag="el", name=f"el{c}")
            nc.scalar.activation(out=el[:], in_=st2[:], func=AF.Exp, scale=SCALE)
            els[c] = el
        # ---------------- node attention scores (exp kept, acc later) -------------
        en_t = []
        for b in range(4):
            st2 = pbig.tile([128, 1024], FP, tag="mm")
            for h in range(2):
                nc.tensor.matmul(st2[:, h * 512:(h + 1) * 512],
                                 kvtn[0:64, b * 128:(b + 1) * 128],
                                 qT[:, h * 512:(h + 1) * 512],
                                 start=True, stop=True)
            en = enp.tile([128, 1024], BF, tag="en", name=f"en{b}")
            nc.scalar.activation(out=en[:], in_=st2[:], func=AF.Exp, scale=SCALE)
            en_t.append(en)

        # ---------------- group-softmax weights over each node's leaf group -------
        s_ps = pbig.tile([16, NC], FP, tag="mm")
        nc.tensor.matmul(s_ps[:], G[:], e_all[:], start=True, stop=True)
        sinv = work.tile([16, NC], FP, tag="sinv")
        nc.vector.reciprocal(sinv[:], s_ps[:])
        r_ps = pbig.tile([128, NC], FP, tag="mm")
        nc.tensor.matmul(r_ps[:], GT[:], sinv[:], start=True, stop=True)
        w_all = work.tile([128, NC], FP, tag="w_all")
        nc.vector.tensor_tensor(out=w_all[:], in0=e_all[:], in1=r_ps[:], op=OP.mult)

        # ---------------- carry fold into last row of each interp chunk -----------
        tot_ps = ptr.tile([NC, 64], FP, tag="tp")
        nc.tensor.transpose(tot_ps[:], totT[:], ident[0:64, 0:64])
        totals = work.tile([NC, 64], FP, tag="tot")
        nc.vector.tensor_copy(totals[:], tot_ps[:])
        carrT_ps = ptr.tile([64, NC], FP, tag="tp")
        nc.tensor.matmul(carrT_ps[:], totals[:], tri32s[:], start=True, stop=True)
        # interpT[f, 128c+127] += carryT[f, c]  (row 127 is in every suffix sum)
        last_rows = interpT[64:128, 127::128]
        nc.vector.tensor_tensor(out=last_rows, in0=last_rows, in1=carrT_ps[:], op=OP.add)

        # One ACT-paced loop fuses: comb chunk build (PE transposes + DVE copy),
        # leaf attention scores+exp, the suffix-mean/node_hat machinery (rides in
        # the exp shadow), and the o2 accumulation (lags one chunk behind its exp).
        # wall32[:, c, :] holds w(l,c)*G placed at a 32-aligned half so chunk
        # pairs can accumulate node_hat at legal PE tile positions with no
        # in-loop pool work.
        comb = big.tile([128, NC, 129], BF)
        nc.gpsimd.memset(comb[:, :, 0:1], 1.0)
        nh_nat = big.tile([128, 4, 65], BF)    # [1 | nh] per node-chunk
        nc.gpsimd.memset(nh_nat[:, :, 0:1], 1.0)
        wall32 = big.tile([128, NC, 32], BF)
        nc.gpsimd.memset(wall32[:], 0.0)
        for c in range(NC):
            o16 = 16 * (c % 2)
            nc.gpsimd.tensor_scalar(out=wall32[:, c, o16:o16 + 16],
                                    in0=G[:], scalar1=w_all[:, c:c + 1],
                                    scalar2=None, op0=OP.mult)
        # o2T accumulates t-major: for each 128-target block k, region
        # [:, k//4, (k%4)*128 : +65] holds [Z2 | o2 vals] with t on partitions.
        o2t_ps = pacc.tile([128, 2, 512], FP, tag="acc", name="o2t_ps")
        for c in range(NC):
            cs = slice(c * 128, (c + 1) * 128)
            tpc = ptr.tile([128, 1024], BF, tag="tp")
            nc.tensor.transpose(tpc[:, 0:64], kvt12[64:128, cs],
                                ident_bf[64:128, 64:128])
            nc.tensor.transpose(tpc[:, 64:128], interpT[64:128, cs],
                                ident_bf[64:128, 64:128])
            nc.vector.tensor_copy(comb[:, c, 1:129], tpc[:, 0:128])
            if c % 4 == 3:
                # suffix-mean for chunks 4c4..4c4+3, then their node_hat partials
                # accumulated in the same PSUM tile (cols 256:320)
                c4 = c // 4
                sfx = pbig.tile([128, 1024], FP, tag="mm", name=f"sfx{c4}")
                nc.tensor.matmul(sfx[:, 0:256].rearrange("p (cc f) -> p cc f", f=64),
                                 tri128[:], comb[:, 4 * c4:4 * c4 + 4, 65:129],
                                 start=True, stop=True)
                upw4 = work.tile([128, 4, 64], BF, tag="upw")
                nc.vector.tensor_tensor(
                    out=upw4[:],
                    in0=sfx[:, 0:256].rearrange("p (cc f) -> p cc f", f=64),
                    in1=_rep_ap(inv3[:, 4 * c4:4 * c4 + 4], 64), op=OP.mult)
                for jc in range(4):
                    cc = 4 * c4 + jc
                    po = 32 * (jc // 2)
                    nc.tensor.matmul(sfx[po:po + 32, 256:320], wall32[:, cc, :],
                                     upw4[:, jc, :],
                                     start=(jc % 2 == 0), stop=(jc % 2 == 1),
                                     skip_group_check=True)
                g, ghalf = c4 // 2, c4 % 2
                nc.vector.tensor_copy(nh_nat[64 * ghalf:64 * ghalf + 64, g, 1:65],
                                      sfx[0:64, 256:320])
            for k in range(T // 128):
                nc.tensor.matmul(
                    o2t_ps[:, k // 4, (k % 4) * 128:(k % 4) * 128 + 65],
                    els[c][:, k * 128:(k + 1) * 128],
                    comb[:, c, 0:65],
                    start=(c == 0), stop=(c == NC - 1), skip_group_check=True)

        # ---------------- node attention accumulate, t-major ----------------------
        # ptr's two transpose slots are free after the merged loop; they hold
        # the two 4-block halves of o1T.
        o1t = [ptr.tile([128, 512], FP, tag="tp", name=f"o1t{a}") for a in range(2)]
        for b in range(4):
            for k in range(T // 128):
                nc.tensor.matmul(o1t[k // 4][:, (k % 4) * 128:(k % 4) * 128 + 65],
                                 en_t[b][:, k * 128:(k + 1) * 128],
                                 nh_nat[:, b, 0:65],
                                 start=(b == 0), stop=(b == 3),
                                 skip_group_check=True)

        # ---------------- combine + final softmax over F, t-major -----------------
        nc.sync.dma_start(root_nat[:], _bcast_ap(d_root[:].rearrange("o f -> (o f)")))
        nc.vector.tensor_scalar(out=root3_nat[:], in0=root_nat[:],
                                scalar1=1.0 / 3.0, scalar2=None, op0=OP.mult)

        def _oview(t, off, n):
            # [128, 8 blocks, n] strided view of the per-t-block regions
            return bass.AP(tensor=t.tensor, offset=t.offset + off,
                           ap=[list(t.ap[0])] + [[512, 2], [128, 4], [1, n]])

        def _rep_mid(ap, rep):
            # [128, rep, ...] view with a step-0 block dim after the partition
            return bass.AP(tensor=ap.tensor, offset=ap.offset,
                           ap=[list(ap.ap[0])] + [[0, rep]] + list(ap.ap)[1:])

        rz1 = work.tile([128, 8], FP, tag="rz1")
        nc.vector.reciprocal(rz1[:].rearrange("p (a b o) -> p a b o", a=2, o=1),
                             _oview(o2t_ps[:], 0, 1))
        def _hview(t, off, n):
            # [128, 4 blocks, n] strided view within one o1t half
            return bass.AP(tensor=t.tensor, offset=t.offset + off,
                           ap=[list(t.ap[0])] + [[128, 4], [1, n]])

        rz2 = work.tile([128, 8], FP, tag="rz2")
        for a in range(2):
            nc.vector.reciprocal(rz2[:, 4 * a:4 * a + 4]
                                 .rearrange("p (b o) -> p b o", o=1),
                                 _hview(o1t[a][:], 0, 1))
        s12all = big.tile([128, 8, F], FP)
        x2all = big.tile([128, 8, F], FP)
        nc.vector.tensor_tensor(out=s12all[:], in0=_oview(o2t_ps[:], 1, 64),
                                in1=_rep_ap(rz1[:], F), op=OP.mult)
        for a in range(2):
            nc.vector.tensor_tensor(out=x2all[:, 4 * a:4 * a + 4, :],
                                    in0=_hview(o1t[a][:], 1, 64),
                                    in1=_rep_ap(rz2[:, 4 * a:4 * a + 4], F),
                                    op=OP.mult)
        nc.gpsimd.tensor_tensor(out=s12all[:], in0=s12all[:], in1=x2all[:], op=OP.add)
        nc.gpsimd.tensor_tensor(out=s12all[:], in0=s12all[:],
                                in1=_rep_mid(root3_nat[:], 8), op=OP.add)
        e3a = big.tile([128, 8, F], FP)
        nc.scalar.activation(out=e3a[:], in_=s12all[:], func=AF.Exp)
        z8 = work.tile([128, 8], FP, tag="z8")
        nc.vector.tensor_reduce(out=z8[:], in_=e3a[:], axis=AX.X, op=OP.add)
        rz = work.tile([128, 8], FP, tag="rz")
        nc.vector.reciprocal(rz[:], z8[:])
        onat = big.tile([128, 8, F], FP)
        nc.vector.tensor_tensor(out=onat[:], in0=e3a[:], in1=_rep_ap(rz[:], F),
                                op=OP.mult)
        nc.sync.dma_start(d_out[:].rearrange("(k p) f -> p k f", p=128), onat[:])


_NC_CACHE = None


def kernel(**inputs):
    global _NC_CACHE
    if _NC_CACHE is None:
        _NC_CACHE = build_nc()
    nc = _NC_CACHE
    shared = {k: np.ascontiguousarray(np.asarray(inputs[k], dtype=np.float32))
              for k in ("Wq", "bq", "Wk", "bk", "Wv", "bv", "Wagg", "bagg")}
    in_maps = []
    for b in range(B):
        m = dict(shared)
        m["root"] = np.ascontiguousarray(np.asarray(inputs["root"][b], dtype=np.float32))
        m["node"] = np.ascontiguousarray(np.asarray(inputs["node"][b], dtype=np.float32))
        m["leaf"] = np.ascontiguousarray(np.asarray(inputs["leaf"][b], dtype=np.float32))
        m["target"] = np.ascontiguousarray(np.asarray(inputs["target"][b], dtype=np.float32))
        in_maps.append(m)
    res = run_bass_kernel_spmd(nc, in_maps, core_ids=list(range(B)))
    return np.stack([r["out"] for r in res.results], axis=0)
